# revision 17
# baseline (speedup 1.0000x reference)
"""DialogueRNNCell Trainium2 kernel (8-core data-parallel over batch).

B=2048 sharded 256/core, parameters replicated.  The heavy work is the 4
"simple attention" heads over two (T=256, B, 256) history slabs.  Per slab
each (t,b,d) tile is streamed from HBM once; PE transposes give d-major
blocks for the score matmuls, and the same natural tiles feed the
weighted-sum matmuls (lhsT = M block, rhs = exp(score) pair) which
accumulate c into persistent PSUM banks.  Positional encoding never
touches the big stream: its score contribution is injected by a tiny
per-chunk init matmul and its context contribution by 2 small matmuls
per chunk.  Small layers (dense_a/b, g/p/e GRUs, self-att, dense1/2) run
in feature-on-partition layout.  Party select/scatter uses one-hot qmask
multiplies.  Host-side work is layout only (sharding, weight/bias
re-tiling, input-independent constants).
"""

import sys

sys.path.insert(0, "/opt/trn_rl_repo")

from contextlib import ExitStack

import numpy as np

import concourse.bass as bass
import concourse.mybir as mybir
from concourse import bacc, tile
from concourse.bass_utils import run_bass_kernel_spmd

F32 = mybir.dt.float32
F32R = mybir.dt.float32r
AF = mybir.ActivationFunctionType
ALU = mybir.AluOpType

N_CORES = 8
B_FULL = 2048
T = 256
P = 2
D_IN_A, D_IN_B = 1024, 512
D_M, D_G, D_P, D_E = 512, 256, 256, 256
B = B_FULL // N_CORES          # 256 per core
CHUNK = 16                     # batch elements per streamed chunk
NCHUNK = B // CHUNK
CCOL = CHUNK * 4               # score-bank cols per chunk: 4*bi + 2*tt + h
NT = T // 128                  # t tiles per slab
BBLK = B // 128                # 128-row batch blocks
PART = 128


def _r(ap):
    return ap.bitcast(F32R)


# ---------------------------------------------------------------- host prep
def _wt_tiles(w):
    """(out_d, in_d) weight -> lhsT SBUF layout (128, kt*out_d)."""
    out_d, in_d = w.shape
    kt = in_d // PART
    return np.ascontiguousarray(
        w.T.reshape(kt, PART, out_d).transpose(1, 0, 2).reshape(PART, kt * out_d)
    ).astype(np.float32)


def _bias_tiles(b):
    n = b.shape[0]
    return np.ascontiguousarray(b.reshape(n // PART, PART).T).astype(np.float32)


def _pe_table():
    pos = np.arange(T, dtype=np.float32)[:, None]
    div = np.exp(np.arange(0, D_G, 2, dtype=np.float32) * (-np.log(10000.0) / D_G))
    pe = np.zeros((T, D_G), np.float32)
    pe[:, 0::2] = np.sin(pos * div)
    pe[:, 1::2] = np.cos(pos * div)
    return pe


def host_constants():
    pe = _pe_table()
    init_pat = np.zeros((NT, 2, CCOL), np.float32)
    for tt in range(NT):
        for h in range(2):
            for bi in range(CHUNK):
                init_pat[tt, h, 4 * bi + 2 * tt + h] = 1.0
    return {
        "c_pe_nat": np.ascontiguousarray(pe.reshape(NT, PART, D_G)),
        "c_pe_t": np.ascontiguousarray(pe.T.reshape(2, PART, T)),
        "c_ident": np.eye(PART, dtype=np.float32),
        "c_ones_row": np.ones((1, PART), np.float32),
        "c_ones_col": np.ones((PART, 1), np.float32),
        "c_initpat": np.ascontiguousarray(init_pat),
    }


def host_params(params):
    def g(a, b):
        return np.asarray(params[a][b], np.float32)

    out = {
        "dense_a_wt": _wt_tiles(g("dense_a", "W")),
        "dense_a_b": _bias_tiles(g("dense_a", "b")),
        "dense_b_wt": _wt_tiles(g("dense_b", "W")),
        "dense_b_b": _bias_tiles(g("dense_b", "b")),
        "dense1_wt": _wt_tiles(g("dense1", "W")),
        "dense1_b": _bias_tiles(g("dense1", "b")),
        "dense2_wt": _wt_tiles(g("dense2", "W")),
        "dense2_b": _bias_tiles(g("dense2", "b")),
    }
    for cell in ("g_cell_a", "g_cell_b", "p_cell_a", "p_cell_b",
                 "e_cell_a", "e_cell_b"):
        out[f"{cell}_wih"] = _wt_tiles(g(cell, "W_ih"))
        out[f"{cell}_whh"] = _wt_tiles(g(cell, "W_hh"))
        out[f"{cell}_bih"] = _bias_tiles(g(cell, "b_ih"))
        out[f"{cell}_bhh"] = _bias_tiles(g(cell, "b_hh"))
    aw = np.stack([g(f"att{i}", "W")[0] for i in (1, 2, 3, 4)], axis=1)
    out["att_w"] = np.ascontiguousarray(aw.reshape(2, PART, 4)).astype(np.float32)
    sw = np.stack([g(f"self_att{i}", "W")[0] for i in (1, 2)], axis=1)
    out["selfatt_w"] = np.ascontiguousarray(sw.reshape(2, PART, 2)).astype(np.float32)
    out["selfatt_b"] = np.array(
        [[g("self_att1", "b")[0], g("self_att2", "b")[0]]], np.float32)
    return out


# ---------------------------------------------------------------- builder
def build(nc: bass.Bass):
    def din(name, shape, dt=F32):
        return nc.dram_tensor(name, list(shape), dt, kind="ExternalInput").ap()

    def dout(name, shape):
        return nc.dram_tensor(name, list(shape), F32, kind="ExternalOutput").ap()

    d_in = {
        "ua": din("ua", (B, D_IN_A)),
        "ub": din("ub", (B, D_IN_B)),
        "qmask": din("qmask", (B, P)),
        "g_hist_a": din("g_hist_a", (T, B, D_G)),
        "g_hist_b": din("g_hist_b", (T, B, D_G)),
        "q0_a": din("q0_a", (B, P, D_P)),
        "q0_b": din("q0_b", (B, P, D_P)),
        "e0_a": din("e0_a", (B, D_E)),
        "e0_b": din("e0_b", (B, D_E)),
        "c_pe_nat": din("c_pe_nat", (NT, PART, D_G), F32R),
        "c_pe_t": din("c_pe_t", (2, PART, T), F32R),
        "c_ident": din("c_ident", (PART, PART)),
        "c_ones_row": din("c_ones_row", (1, PART)),
        "c_ones_col": din("c_ones_col", (PART, 1), F32R),
        "c_initpat": din("c_initpat", (NT, 2, CCOL), F32R),
        "att_w": din("att_w", (2, PART, 4), F32R),
        "selfatt_w": din("selfatt_w", (2, PART, 2)),
        "selfatt_b": din("selfatt_b", (1, 2)),
        "dense_a_wt": din("dense_a_wt", (PART, 8 * D_M), F32R),
        "dense_a_b": din("dense_a_b", (PART, 4)),
        "dense_b_wt": din("dense_b_wt", (PART, 4 * D_M), F32R),
        "dense_b_b": din("dense_b_b", (PART, 4)),
        "dense1_wt": din("dense1_wt", (PART, 4 * D_G)),
        "dense1_b": din("dense1_b", (PART, 2)),
        "dense2_wt": din("dense2_wt", (PART, 4 * D_G)),
        "dense2_b": din("dense2_b", (PART, 2)),
    }
    for cell, ind in (("g_cell_a", D_M + D_P), ("g_cell_b", D_M + D_P),
                      ("p_cell_a", D_M + D_G), ("p_cell_b", D_M + D_G),
                      ("e_cell_a", D_P), ("e_cell_b", D_P)):
        d_in[f"{cell}_wih"] = din(f"{cell}_wih", (PART, (ind // PART) * 3 * D_G), F32R)
        d_in[f"{cell}_whh"] = din(f"{cell}_whh", (PART, (D_G // PART) * 3 * D_G))
        d_in[f"{cell}_bih"] = din(f"{cell}_bih", (PART, 6))
        d_in[f"{cell}_bhh"] = din(f"{cell}_bhh", (PART, 6))

    d_out = {
        "g_a": dout("g_a", (B, D_G)),
        "g_b": dout("g_b", (B, D_G)),
        "q_a": dout("q_a", (B, P, D_P)),
        "q_b": dout("q_b", (B, P, D_P)),
        "e_a": dout("e_a", (B, D_E)),
        "e_b": dout("e_b", (B, D_E)),
        "alpha": dout("alpha", (B, 1, T)),
    }

    rr = [0]

    def copy_ps(dst_ap, src_ap, rnd=False, eng=None):
        """PSUM->SBUF copy on ACT (eng=0) or DVE (eng=1).

        Keep `eng` deterministic per PSUM slot so each slot has a single
        consumer engine (keeps matmul sync-wait fan-in low).  rnd=True
        writes float32r so the output is legal as an fp32r matmul operand.
        """
        if rnd:
            dst_ap = _r(dst_ap)
        if eng is None:
            rr[0] += 1
            eng = rr[0] % 2
        if eng == 0:
            nc.scalar.copy(dst_ap, src_ap)
        else:
            nc.vector.tensor_copy(dst_ap, src_ap)

    I32 = mybir.dt.int32

    def mm_transpose(out_ap, in_ap, ident_ap):
        """out = in^T via a normal matmul (exact; carries >1 sync wait,
        unlike transpose-mode whose LW uop only fits one)."""
        nc.tensor.matmul(out_ap, in_ap, ident_ap, start=True, stop=True)

    def select(out_ap, mask_ap, on_true, on_false):
        nc.vector.tensor_copy(out_ap, on_false)
        nc.vector.copy_predicated(out_ap, mask_ap.bitcast(I32), on_true)

    with tile.TileContext(nc) as tc, ExitStack() as top:
        const = top.enter_context(tc.tile_pool(name="const", bufs=1))
        persist = top.enter_context(tc.tile_pool(name="persist", bufs=1))

        ident = const.tile([PART, PART], F32, tag="ident")
        nc.sync.dma_start(ident[:], d_in["c_ident"][:])
        identr = const.tile([PART, PART], F32R, tag="identr")
        nc.sync.dma_start(identr[:], _r(d_in["c_ident"][:]))
        ones_row = const.tile([1, PART], F32, tag="ones_row")
        nc.sync.dma_start(ones_row[:], d_in["c_ones_row"][:])
        ones_col = const.tile([PART, 1], F32R, tag="ones_col")
        nc.sync.dma_start(ones_col[:], d_in["c_ones_col"][:])
        sa_b = const.tile([1, 2], F32, tag="sa_b")
        nc.sync.dma_start(sa_b[:], d_in["selfatt_b"][:])
        pe_nat = const.tile([PART, NT * D_G], F32R, tag="pe_nat")
        pe_t = const.tile([PART, 2 * T], F32R, tag="pe_t")
        initpat = const.tile([2, NT * CCOL], F32R, tag="initpat")
        att_w = const.tile([PART, 8], F32R, tag="att_w")
        sa_w = const.tile([PART, 4], F32, tag="sa_w")
        for tt in range(NT):
            nc.sync.dma_start(pe_nat[:, tt * D_G:(tt + 1) * D_G],
                              d_in["c_pe_nat"][tt])
            nc.sync.dma_start(initpat[:, tt * CCOL:(tt + 1) * CCOL],
                              d_in["c_initpat"][tt])
        for j in range(2):
            nc.sync.dma_start(pe_t[:, j * T:(j + 1) * T], d_in["c_pe_t"][j])
            nc.sync.dma_start(att_w[:, j * 4:(j + 1) * 4], d_in["att_w"][j])
            nc.sync.dma_start(sa_w[:, j * 2:(j + 1) * 2], d_in["selfatt_w"][j])

        # ---------------- shared emit helpers
        def gru(psum_pool, sbuf_pool, name, wih, whh, bih, bhh, x_tiles,
                h_tiles):
            """torch GRUCell (hidden 256) -> 2 new feature tiles (128, B)."""
            nb = B
            H3 = 3 * D_G
            bsum = sbuf_pool.tile([PART, 6], F32, tag="gru_bsum")
            nc.vector.tensor_add(bsum[:], bih[:], bhh[:])
            rz = sbuf_pool.tile([PART, 4 * nb], F32, tag="gru_rz")
            for j in range(4):        # r tiles 0..1, z tiles 2..3
                ps = psum_pool.tile([PART, nb], F32, tag="lin_ps", bufs=2)
                for i, xt in enumerate(x_tiles):
                    w_sl = wih[:, i * H3 + j * PART: i * H3 + (j + 1) * PART]
                    if xt.dtype == F32R:
                        nc.tensor.matmul(ps[:], w_sl, xt,
                                         start=(i == 0), stop=False)
                    else:
                        nc.tensor.matmul(ps[:], w_sl.bitcast(F32), xt,
                                         start=(i == 0), stop=False)
                for i, ht in enumerate(h_tiles):
                    nc.tensor.matmul(
                        ps[:],
                        whh[:, i * H3 + j * PART: i * H3 + (j + 1) * PART],
                        ht, start=False, stop=(i == len(h_tiles) - 1))
                nc.scalar.activation(rz[:, j * nb:(j + 1) * nb], ps[:],
                                     AF.Sigmoid, bias=bsum[:, j: j + 1])
            hnew = []
            for j in range(2):
                jj = 4 + j
                ps_i = psum_pool.tile([PART, nb], F32, tag="lin_psi", bufs=1)
                for i, xt in enumerate(x_tiles):
                    w_sl = wih[:, i * H3 + jj * PART: i * H3 + (jj + 1) * PART]
                    if xt.dtype == F32R:
                        nc.tensor.matmul(ps_i[:], w_sl, xt, start=(i == 0),
                                         stop=(i == len(x_tiles) - 1))
                    else:
                        nc.tensor.matmul(ps_i[:], w_sl.bitcast(F32), xt,
                                         start=(i == 0),
                                         stop=(i == len(x_tiles) - 1))
                ps_h = psum_pool.tile([PART, nb], F32, tag="lin_ps2", bufs=2)
                for i, ht in enumerate(h_tiles):
                    nc.tensor.matmul(
                        ps_h[:],
                        whh[:, i * H3 + jj * PART: i * H3 + (jj + 1) * PART],
                        ht, start=(i == 0), stop=(i == len(h_tiles) - 1))
                hn = sbuf_pool.tile([PART, nb], F32, tag="gru_hn")
                nc.scalar.activation(hn[:], ps_h[:], AF.Identity,
                                     bias=bhh[:, jj: jj + 1])
                nc.vector.tensor_mul(hn[:], rz[:, j * nb:(j + 1) * nb], hn[:])
                ng = sbuf_pool.tile([PART, nb], F32, tag="gru_n")
                nc.vector.scalar_tensor_tensor(
                    out=ng[:], in0=ps_i[:], scalar=bih[:, jj: jj + 1],
                    in1=hn[:], op0=ALU.add, op1=ALU.add)
                nc.scalar.activation(ng[:], ng[:], AF.Tanh)
                z_sl = rz[:, (2 + j) * nb:(3 + j) * nb]
                ho = sbuf_pool.tile([PART, nb], F32, tag=f"gru_o{j}")
                nc.vector.tensor_sub(ho[:], h_tiles[j], ng[:])
                nc.vector.tensor_mul(ho[:], ho[:], z_sl)
                nc.vector.tensor_add(ho[:], ho[:], ng[:])
                hnew.append(ho)
            return hnew

        def store_nat(dram_rows, feat_tiles, psum_pool, nat_pool, ncols):
            """feature tiles -> natural (B, ncols) DRAM rows."""
            for bb in range(BBLK):
                nat = nat_pool.tile([PART, ncols], F32, tag="nat_out")
                for j, ft in enumerate(feat_tiles):
                    pt = psum_pool.tile([PART, PART], F32, tag="tp", bufs=2)
                    mm_transpose(
                        pt[:], ft[:, bb * PART:(bb + 1) * PART], ident[:])
                    copy_ps(nat[:, j * PART:(j + 1) * PART], pt[:],
                            eng=(bb + j) % 2)
                nc.sync.dma_start(dram_rows[bb * PART:(bb + 1) * PART], nat[:])

        # ============================================================
        # Phase 1: inputs -> feature layout; dense_a/b; qmask; g cells
        # ============================================================
        with tc.tile_pool(name="pro_sbuf", bufs=2) as pro, \
             tc.tile_pool(name="pro_w", bufs=1) as prow, \
             tc.tile_pool(name="pro_ps", bufs=2, space="PSUM") as props:

            def load_feat(dram_rows, rows, cols, tag, dst_pool=persist,
                          rnd=False):
                kt = cols // PART
                dst = dst_pool.tile([PART, kt * rows], F32, tag=tag)
                for bb in range(rows // PART):
                    nat = pro.tile([PART, cols], F32, tag="nat_in")
                    nc.sync.dma_start(nat[:], dram_rows[bb * PART:(bb + 1) * PART])
                    for j in range(kt):
                        pt = props.tile([PART, PART], F32, tag="tp", bufs=2)
                        mm_transpose(
                            pt[:], nat[:, j * PART:(j + 1) * PART], ident[:])
                        copy_ps(
                            dst[:, j * rows + bb * PART: j * rows + (bb + 1) * PART],
                            pt[:], rnd=rnd, eng=0)
                return [dst[:, j * rows:(j + 1) * rows] for j in range(kt)]

            def wload(name, tag, pool=prow):
                shp = d_in[name].shape
                t = pool.tile(list(shp), d_in[name].dtype, tag=tag)
                nc.sync.dma_start(t[:], d_in[name][:])
                return t

            ua_t = load_feat(d_in["ua"], B, D_IN_A, "ua_t", dst_pool=pro, rnd=True)
            ub_t = load_feat(d_in["ub"], B, D_IN_B, "ub_t", dst_pool=pro, rnd=True)
            q0_t = {}
            for s in ("a", "b"):
                for p in range(P):
                    q0_t[(s, p)] = load_feat(d_in[f"q0_{s}"][:, p], B, D_P,
                                             f"q0{s}{p}_t")
            e0_t = {s: load_feat(d_in[f"e0_{s}"], B, D_E, f"e0{s}_t")
                    for s in ("a", "b")}
            h_t = {s: load_feat(d_in[f"g_hist_{s}"][T - 1], B, D_G,
                                f"h{s}_t", dst_pool=pro)
                   for s in ("a", "b")}

            # qmask rows + one-hot broadcast tiles
            qm_nat = pro.tile([PART, P * BBLK], F32, tag="qm_nat")
            for bb in range(BBLK):
                nc.sync.dma_start(
                    qm_nat[:, bb * P:(bb + 1) * P],
                    d_in["qmask"][bb * PART:(bb + 1) * PART])
            qm_rows = persist.tile([P, B], F32, tag="qm_rows")
            for bb in range(BBLK):
                pt = props.tile([PART, PART], F32, tag="tp", bufs=2)
                mm_transpose(
                    pt[:P, :], qm_nat[:, bb * P:(bb + 1) * P], ident[:])
                nc.scalar.copy(qm_rows[:, bb * PART:(bb + 1) * PART], pt[:P, :])
            mask_b = []
            for p in range(P):
                qrow = persist.tile([1, B], F32, tag=f"qm_row{p}")
                nc.sync.dma_start(qrow[:], qm_rows[p: p + 1, :])
                ps = props.tile([PART, B], F32, tag="mb_ps", bufs=1)
                nc.tensor.matmul(ps[:], ones_row[:],
                                 qrow[:], start=True, stop=True)
                mb = persist.tile([PART, B], F32, tag=f"mask_b{p}")
                copy_ps(mb[:], ps[:], eng=0)
                mask_b.append(mb)

            # dense_a / dense_b (U features kept for the p cells)
            u_feat = {}
            for s, nkt, xt in (("a", 8, ua_t), ("b", 4, ub_t)):
                wt = wload(f"dense_{s}_wt", f"d{s}_wt")
                bi_ = wload(f"dense_{s}_b", f"d{s}_b")
                dst = persist.tile([PART, 4 * B], F32, tag=f"u{s}_d")
                for j in range(4):
                    ps = props.tile([PART, B], F32, tag="lin_ps", bufs=2)
                    for i, x in enumerate(xt):
                        nc.tensor.matmul(
                            ps[:],
                            wt[:, i * D_M + j * PART: i * D_M + (j + 1) * PART],
                            _r(x), start=(i == 0), stop=(i == len(xt) - 1))
                    nc.scalar.activation(_r(dst[:, j * B:(j + 1) * B]), ps[:],
                                         AF.Identity, bias=bi_[:, j: j + 1])
                u_feat[s] = [_r(dst[:, j * B:(j + 1) * B]) for j in range(4)]

            # g cells + store g outputs
            for s in ("a", "b"):
                q0sel = []
                for j in range(2):
                    t_ = pro.tile([PART, B], F32, tag=f"q0sel{j}")
                    select(t_[:], mask_b[1][:],
                                     q0_t[(s, 1)][j], q0_t[(s, 0)][j])
                    q0sel.append(t_[:])
                wih = wload(f"g_cell_{s}_wih", f"gw{s}i")
                whh = wload(f"g_cell_{s}_whh", f"gw{s}h")
                bih = wload(f"g_cell_{s}_bih", f"gb{s}i")
                bhh = wload(f"g_cell_{s}_bhh", f"gb{s}h")
                gout = gru(props, pro, f"g{s}", wih, whh, bih, bhh,
                           list(u_feat[s]) + q0sel,
                           [h for h in h_t[s]])
                store_nat(d_out[f"g_{s}"], [g_[:] for g_ in gout], props, pro,
                          D_G)

        # ============================================================
        # Phase 2: attention slabs
        # ============================================================
        c_norm = {}      # (slab, dblk) -> SBUF (128, 2B) cols 2b+h
        alpha_sl = {}    # (slab, tt) -> SBUF (128, B)
        for si, s in enumerate(("a", "b")):
            g_dram = d_in[f"g_hist_{s}"]
            with ExitStack() as sl:
                chp = sl.enter_context(tc.tile_pool(name=f"ch_{s}", bufs=2))
                mtp = sl.enter_context(tc.tile_pool(name=f"mt_{s}", bufs=6))
                stp = sl.enter_context(
                    tc.tile_pool(name=f"st_{s}", bufs=2, space="PSUM"))
                scp = sl.enter_context(
                    tc.tile_pool(name=f"sc_{s}", bufs=2, space="PSUM"))
                wsp = sl.enter_context(
                    tc.tile_pool(name=f"ws_{s}", bufs=1, space="PSUM"))
                zps = sl.enter_context(
                    tc.tile_pool(name=f"zp_{s}", bufs=2, space="PSUM"))
                aux = sl.enter_context(tc.tile_pool(name=f"aux_{s}", bufs=1))

                # pe score bias, transposed to (head, t_local) per tt
                pes_T = []
                for tt in range(NT):
                    ps = zps.tile([PART, 512], F32, tag="misc_ps")
                    for j in range(2):
                        nc.tensor.matmul(
                            ps[:, :2],
                            pe_t[:, j * T + tt * PART: j * T + (tt + 1) * PART],
                            att_w[:, j * 4 + 2 * si: j * 4 + 2 * si + 2],
                            start=(j == 0), stop=(j == 1))
                    pes_sb = aux.tile([PART, 2], F32, tag=f"pes{tt}")
                    nc.scalar.copy(pes_sb[:], ps[:, :2])
                    pt = zps.tile([PART, 512], F32, tag="misc_ps")
                    mm_transpose(pt[:2, :PART], pes_sb[:], ident[:])
                    pes_t_sb = aux.tile([2, PART], F32, tag=f"pesT{tt}")
                    nc.scalar.copy(_r(pes_t_sb[:]), pt[:2, :PART])
                    pes_T.append(pes_t_sb)

                wsum = [wsp.tile([PART, 2 * B], F32, tag=f"wsum{j}",
                                 name=f"wsum{j}")
                        for j in range(2)]
                warm = stp.tile([PART, 512], F32, tag="stage")
                nc.tensor.transpose(_r(warm[:, :PART]), identr[:], identr[:])
                e_slab = aux.tile([PART, NCHUNK * CCOL], F32, tag="e_slab")
                z_slab = aux.tile([1, NCHUNK * CCOL], F32, tag="z_slab")

                for ci in range(NCHUNK):
                    c0 = ci * CHUNK
                    ch = []
                    for tt in range(NT):
                        t_ = chp.tile([PART, CHUNK, D_G], F32R, tag=f"ch{tt}")
                        nc.sync.dma_start(
                            t_[:], _r(g_dram[tt * PART:(tt + 1) * PART,
                                             c0:c0 + CHUNK]))
                        ch.append(t_)
                    sc = scp.tile([PART, CCOL], F32, tag="scorebank")
                    for tt in range(NT):
                        nc.tensor.matmul(
                            sc[:], _r(pes_T[tt][:]),
                            initpat[:, tt * CCOL:(tt + 1) * CCOL],
                            start=(tt == 0), stop=False)
                    for bi in range(CHUNK):
                        mt = mtp.tile([PART, 512], F32, tag="mtile")
                        st = stp.tile([PART, 512], F32, tag="stage")
                        for tt in range(NT):
                            for j in range(2):
                                nc.tensor.transpose(
                                    _r(st[:, (2 * tt + j) * PART:
                                          (2 * tt + j + 1) * PART]),
                                    ch[tt][:, bi, j * PART:(j + 1) * PART],
                                    identr[:])
                        copy_ps(mt[:], st[:], rnd=True, eng=bi % 2)
                        for tt in range(NT):
                            for j in range(2):
                                nc.tensor.matmul(
                                    sc[:, 4 * bi + 2 * tt: 4 * bi + 2 * tt + 2],
                                    _r(mt[:, (2 * tt + j) * PART:
                                          (2 * tt + j + 1) * PART]),
                                    att_w[:, j * 4 + 2 * si:
                                          j * 4 + 2 * si + 2],
                                    start=False,
                                    stop=(tt == NT - 1 and j == 1))
                    e_sl = e_slab[:, ci * CCOL:(ci + 1) * CCOL]
                    nc.scalar.activation(_r(e_sl), sc[:], AF.Exp)
                    zp = zps.tile([PART, 512], F32, tag="misc_ps")
                    nc.tensor.matmul(zp[:1, :CCOL], ones_col[:], _r(e_sl),
                                     start=True, stop=True)
                    nc.scalar.copy(z_slab[:, ci * CCOL:(ci + 1) * CCOL],
                                   zp[:1, :CCOL])
                    for bi in range(CHUNK):
                        for tt in range(NT):
                            ecols = e_slab[:, ci * CCOL + 4 * bi + 2 * tt:
                                           ci * CCOL + 4 * bi + 2 * tt + 2]
                            for j in range(2):
                                nc.tensor.matmul(
                                    wsum[j][:, 2 * (c0 + bi): 2 * (c0 + bi) + 2],
                                    ch[tt][:, bi, j * PART:(j + 1) * PART],
                                    _r(ecols), start=(tt == 0), stop=False)
                    e_chunk = e_slab[:, ci * CCOL:(ci + 1) * CCOL].rearrange(
                        "p (b t h) -> p b t h", t=2, h=2)
                    for j in range(2):
                        for tt in range(NT):
                            nc.tensor.matmul(
                                wsum[j][:, 2 * c0: 2 * (c0 + CHUNK)],
                                pe_nat[:, tt * D_G + j * PART:
                                       tt * D_G + (j + 1) * PART],
                                _r(e_chunk[:, :, tt, :]),
                                start=False, stop=(tt == NT - 1))

                # normalize c and alpha
                zr = aux.tile([1, 2 * B], F32, tag="zr")
                zv = z_slab[:].rearrange("p (b t h) -> p b t h", t=2, h=2)
                zrv = zr[:].rearrange("p (b h) -> p b h", h=2)
                for h in range(2):
                    nc.vector.tensor_add(zrv[:, :, h], zv[:, :, 0, h],
                                         zv[:, :, 1, h])
                nc.vector.reciprocal(zr[:], zr[:])
                zb_ps = zps.tile([PART, 512], F32, tag="misc_ps")
                nc.tensor.matmul(zb_ps[:, :2 * B], ones_row[:], zr[:],
                                 start=True, stop=True)
                zb = aux.tile([PART, 2 * B], F32, tag="zb")
                copy_ps(zb[:], zb_ps[:, :2 * B], eng=1)
                for j in range(2):
                    cn = persist.tile([PART, 2 * B], F32, tag=f"c_{s}_{j}")
                    nc.vector.tensor_mul(cn[:], wsum[j][:], zb[:])
                    c_norm[(s, j)] = cn
                ev = e_slab[:].rearrange("p (b t h) -> p b t h", t=2, h=2)
                zbv = zb[:].rearrange("p (b h) -> p b h", h=2)
                for tt in range(NT):
                    a_t = persist.tile([PART, B], F32, tag=f"alpha_{s}{tt}")
                    nc.vector.tensor_mul(a_t[:], ev[:, :, tt, 0], zbv[:, :, 0])
                    tmp = aux.tile([PART, B], F32, tag="atmp")
                    nc.vector.tensor_mul(tmp[:], ev[:, :, tt, 1], zbv[:, :, 1])
                    nc.vector.tensor_add(a_t[:], a_t[:], tmp[:])
                    alpha_sl[(s, tt)] = a_t

        # ============================================================
        # Phase 3: alpha out, self-att + dense1/2, p cells, e cells
        # ============================================================
        with tc.tile_pool(name="tail_sbuf", bufs=2) as tl, \
             tc.tile_pool(name="tail_w", bufs=1) as tlw, \
             tc.tile_pool(name="tail_ps", bufs=2, space="PSUM") as tps:

            for bb in range(BBLK):
                nat = tl.tile([PART, T], F32, tag="alpha_nat")
                for tt in range(NT):
                    asum = tl.tile([PART, PART], F32, tag="alpha_sum")
                    nc.vector.tensor_add(
                        asum[:],
                        alpha_sl[("a", tt)][:, bb * PART:(bb + 1) * PART],
                        alpha_sl[("b", tt)][:, bb * PART:(bb + 1) * PART])
                    pt = tps.tile([PART, PART], F32, tag="tp", bufs=2)
                    mm_transpose(pt[:], asum[:], ident[:])
                    copy_ps(nat[:, tt * PART:(tt + 1) * PART], pt[:],
                            eng=(bb + tt) % 2)
                nc.sync.dma_start(
                    d_out["alpha"][bb * PART:(bb + 1) * PART, 0], nat[:])

            for si, s in enumerate(("a", "b")):
                # self attention over [m0, m1] (N=2), 'general2' variant
                if s == "a":
                    srcs = [("a", 0), ("b", 0)]      # c_aa, c_ab
                else:
                    srcs = [("b", 1), ("a", 1)]      # c_bb, c_ba
                m_t = []
                for (sl_, h) in srcs:
                    m_t.append([
                        c_norm[(sl_, j)][:].rearrange(
                            "p (b h) -> p b h", h=2)[:, :, h]
                        for j in range(2)])
                sc_e = []
                for n in range(2):
                    ps = tps.tile([PART, B], F32, tag="sa", bufs=1)
                    for j in range(2):
                        nc.tensor.matmul(
                            ps[:1, :], sa_w[:, j * 2 + si: j * 2 + si + 1],
                            m_t[n][j], start=(j == 0), stop=(j == 1))
                    e_ = tl.tile([1, B], F32, tag=f"sa_e{n}")
                    nc.scalar.activation(e_[:], ps[:1, :], AF.Exp,
                                         bias=sa_b[:, si: si + 1])
                    sc_e.append(e_)
                ssum = tl.tile([1, B], F32, tag="sa_sum")
                nc.vector.tensor_add(ssum[:], sc_e[0][:], sc_e[1][:])
                nc.vector.reciprocal(ssum[:], ssum[:])
                x_sa = tl.tile([PART, 4 * B], F32, tag="x_sa")
                for n in range(2):
                    an = tl.tile([1, B], F32, tag=f"sa_a{n}")
                    nc.vector.tensor_mul(an[:], sc_e[n][:], ssum[:])
                    ab_ps = tps.tile([PART, B], F32, tag="sa", bufs=1)
                    nc.tensor.matmul(ab_ps[:], ones_row[:], an[:],
                                     start=True, stop=True)
                    ab = tl.tile([PART, B], F32, tag=f"sa_ab{n}")
                    copy_ps(ab[:], ab_ps[:], eng=0)
                    for j in range(2):
                        nc.vector.tensor_mul(
                            x_sa[:, (2 * n + j) * B:(2 * n + j + 1) * B],
                            m_t[n][j], ab[:])
                dwt = tlw.tile([PART, 4 * D_G], F32, tag=f"d12w{s}")
                nc.sync.dma_start(dwt[:], d_in[f"dense{si + 1}_wt"][:])
                dbs = tlw.tile([PART, 2], F32, tag=f"d12b{s}")
                nc.sync.dma_start(dbs[:], d_in[f"dense{si + 1}_b"][:])
                cfin = tl.tile([PART, 2 * B], F32, tag=f"cfin{s}")
                for j in range(2):
                    ps = tps.tile([PART, B], F32, tag="lin_ps", bufs=2)
                    for i in range(4):
                        nc.tensor.matmul(
                            ps[:],
                            dwt[:, i * D_G + j * PART: i * D_G + (j + 1) * PART],
                            x_sa[:, i * B:(i + 1) * B],
                            start=(i == 0), stop=(i == 3))
                    nc.scalar.activation(_r(cfin[:, j * B:(j + 1) * B]), ps[:],
                                         AF.Identity, bias=dbs[:, j: j + 1])
                c_tiles = [_r(cfin[:, j * B:(j + 1) * B]) for j in range(2)]

                def wl(name, tag):
                    t_ = tlw.tile(list(d_in[name].shape), d_in[name].dtype,
                                  tag=tag)
                    nc.sync.dma_start(t_[:], d_in[name][:])
                    return t_

                pwih = wl(f"p_cell_{s}_wih", "pwi")
                pwhh = wl(f"p_cell_{s}_whh", "pwh")
                pbih = wl(f"p_cell_{s}_bih", "pbi")
                pbhh = wl(f"p_cell_{s}_bhh", "pbh")
                x_p = list(u_feat[s]) + c_tiles
                q_new = {}
                for p in range(P):
                    qs = gru(tps, tl, f"p{s}{p}", pwih, pwhh, pbih, pbhh,
                             x_p, q0_t[(s, p)])
                    qp = []
                    for j in range(2):
                        t_ = tl.tile([PART, B], F32, tag=f"qnew{p}{j}")
                        select(t_[:], mask_b[p][:], qs[j][:],
                                         q0_t[(s, p)][j])
                        qp.append(t_)
                    q_new[p] = qp
                    store_nat(d_out[f"q_{s}"][:, p], [q_[:] for q_ in qp],
                              tps, tl, D_P)
                qsel = []
                for j in range(2):
                    t_ = tl.tile([PART, B], F32, tag=f"qsel{j}")
                    select(t_[:], mask_b[1][:], q_new[1][j][:],
                                     q_new[0][j][:])
                    qsel.append(t_[:])
                ewih = wl(f"e_cell_{s}_wih", "ewi")
                ewhh = wl(f"e_cell_{s}_whh", "ewh")
                ebih = wl(f"e_cell_{s}_bih", "ebi")
                ebhh = wl(f"e_cell_{s}_bhh", "ebh")
                enew = gru(tps, tl, f"e{s}", ewih, ewhh, ebih, ebhh,
                           qsel, e0_t[s])
                store_nat(d_out[f"e_{s}"], [e_[:] for e_ in enew], tps, tl,
                          D_E)


# ---------------------------------------------------------------- runner
_CACHE = {}


def _get_nc():
    if "nc" not in _CACHE:
        nc = bacc.Bacc("TRN2", target_bir_lowering=False, debug=False,
                       num_devices=N_CORES)
        build(nc)
        nc.compile()
        _CACHE["nc"] = nc
    return _CACHE["nc"]


def kernel(Ua, Ub, qmask, g_hist_a, g_hist_b, q0_a, q0_b, e0_a, e0_b,
           params, k=1, **_ignored):
    nc = _get_nc()
    consts = host_constants()
    pp = host_params(params)

    f32 = lambda x: np.ascontiguousarray(np.asarray(x), dtype=np.float32)
    in_maps = []
    for c in range(N_CORES):
        sl = slice(c * B, (c + 1) * B)
        m = {
            "ua": f32(Ua[sl]),
            "ub": f32(Ub[sl]),
            "qmask": f32(qmask[sl]),
            "g_hist_a": f32(g_hist_a[:, sl]),
            "g_hist_b": f32(g_hist_b[:, sl]),
            "q0_a": f32(q0_a[sl]),
            "q0_b": f32(q0_b[sl]),
            "e0_a": f32(e0_a[sl]),
            "e0_b": f32(e0_b[sl]),
        }
        m.update(consts)
        m.update(pp)
        in_maps.append(m)

    import os
    trace = os.environ.get("KBENCH_TRACE") == "1"
    res = run_bass_kernel_spmd(nc, in_maps, list(range(N_CORES)),
                               trace=trace,
                               tmpdir=os.environ.get("KBENCH_TRACE_DIR") or None)
    _CACHE["last"] = res
    outs = res.results
    cat = lambda name: np.concatenate([outs[c][name] for c in range(N_CORES)],
                                      axis=0)
    return (cat("g_a"), cat("q_a"), cat("e_a"),
            cat("g_b"), cat("q_b"), cat("e_b"), cat("alpha"))
